# revision 62
# baseline (speedup 1.0000x reference)
"""Trainium2 Bass kernel for the segment-reduce masked-CE loss (nn_NewLoss).

Reference math (N=64, C=46, P=2048, MP=256):
    assignment[n, p] = 1 + (p * MP) // P  (contiguous segments of 8 frames)
    pooled[n, q, c]  = mean over the 8 frames of segment q of input[n, c, :]
    loss = -sum_{n,q} lab_mask[n,q] * log_softmax(pooled)[n, q, target[n,q]]

Sharding: data-parallel over batch n across 8 cores (8 items per core);
each core returns per-q partial sums, reduced on the host.

Per-core layout: frames on partitions so the PE does the pooling.
x is shipped fp8_e4m3 as xT[frame, row] (row = 46*item + ch), 16 blocks
of 128 frames; one fp8 DoubleRow matmul per block pair (2 k-tiles), its
32 segments steered into partition band 32*(b2%4) of a full 128-partition
output by the weight layout (DoubleRow dst must start at partition 0), 4
pairs accumulating per PSUM tile S[q, row].  Input DMA is striped across
the three DMA-capable rings (per-ring bandwidth ~60-180 GB/s is the real
limit): scalar carries the weights first (unblocks matmul 0) plus leading
pairs, sync/gpsimd the rest.  Epilogue per tile: picked via
scalar_tensor_tensor accumulate against a host-built masked one-hot
(DVE), EXP (scalar) -> window-46 reduce (DVE) -> Ln (scalar, same act
table as Exp, so no mid-kernel table load) -> mask STT. Host sums the
[128, 4] per-core partials.
"""

import numpy as np

import concourse.bacc as bacc
import concourse.tile as tile
from concourse import mybir
from concourse.bass_utils import run_bass_kernel_spmd

F32 = mybir.dt.float32
BF16 = mybir.dt.bfloat16
F8 = mybir.dt.float8e4

N, C, P, MP = 64, 46, 2048, 256
NCORES = 8
NLOC = N // NCORES            # 8 batch items per core
ROWS = NLOC * C               # 368 (item, channel) rows per core
W = P // MP                   # 8-frame pooling window
NBLK = P // 128               # 16 frame blocks of 128
NPAIR = NBLK // 2             # 8 DoubleRow block pairs
XCOLS = NPAIR * 2 * ROWS      # 5888
CW = 2 * ROWS                 # 736 x cols per pair

# Single combined Exp+Ln activation table: drop Exp/Ln from the per-func
# tables so the fixpoint pass lands on natural_log_exp_and_others and the
# kernel pays only one ACT_TABLE_LOAD (overlapped with the input DMA).
_ORIG_GAT = bacc.get_activation_tables


def _gat_combined(arch):
    exp = mybir.ActivationFunctionType.Exp
    ln = mybir.ActivationFunctionType.Ln
    out = {}
    for name, funcs in _ORIG_GAT(arch).items():
        if name != "natural_log_exp_and_others":
            funcs = funcs - {exp, ln}
        out[name] = funcs
    return out


bacc.get_activation_tables = _gat_combined


def _build_nc():
    nc = bacc.Bacc("TRN2", target_bir_lowering=False)

    x_d = nc.dram_tensor("x", [128, XCOLS], F8, kind="ExternalInput")
    w_d = nc.dram_tensor("w", [128, 352], F8, kind="ExternalInput")
    oh_d = nc.dram_tensor("oh", [128, 2 * ROWS], BF16, kind="ExternalInput")
    mk_d = nc.dram_tensor("mk", [128, 2 * NLOC], F32, kind="ExternalInput")
    loss_d = nc.dram_tensor("loss", [128, 4], F32, kind="ExternalOutput")

    mult = mybir.AluOpType.mult

    with tile.TileContext(nc) as tc:
        with (
            tc.tile_pool(name="xin", bufs=1) as xin,
            tc.tile_pool(name="sb", bufs=1) as sb,
            tc.tile_pool(name="psum", bufs=1, space="PSUM") as psum,
        ):
            # Per-ring DMA bandwidth (~60-110 GB/s) is the real limit, so
            # stripe across all three DMA-capable rings: scalar carries the
            # tiny weights first (unblocks matmul 0) then the leading x
            # pairs, sync and gpsimd carry the rest, oh/mk trail on scalar.
            w8 = sb.tile([128, 352], F8)
            nc.sync.dma_start(out=w8[:], in_=w_d[:])
            x8 = xin.tile([128, XCOLS], F8)
            # one pair per ring in consumption order: each pair has the
            # fewest possible bytes queued ahead of it on its ring, so a
            # slow-phase ring crawl degrades the stream gracefully.
            # pair 0 is split across the scalar and gpsimd cold-starts so
            # matmul 0's gate is half a pair transfer.
            nc.scalar.dma_start(out=x8[:, 0 : CW // 2], in_=x_d[:, 0 : CW // 2])
            nc.gpsimd.dma_start(
                out=x8[:, CW // 2 : CW], in_=x_d[:, CW // 2 : CW]
            )
            stripes = [
                (nc.sync, 1, 2),
                (nc.scalar, 2, 3),
                (nc.gpsimd, 3, 4),
                (nc.sync, 4, 5),
                (nc.scalar, 5, 6),
                (nc.gpsimd, 6, 8),
            ]
            for eng, p0, p1 in stripes:
                eng.dma_start(
                    out=x8[:, p0 * CW : p1 * CW], in_=x_d[:, p0 * CW : p1 * CW]
                )
            oh = sb.tile([128, 2 * ROWS], BF16)
            nc.scalar.dma_start(out=oh[:], in_=oh_d[:])
            mk = sb.tile([128, 2 * NLOC], F32)
            nc.scalar.dma_start(out=mk[:], in_=mk_d[:])

            res = sb.tile([128, 4], F32)

            # DoubleRow pooling: each matmul covers one block pair (2
            # k-tiles of 128 frames); its 32 segments land in partition
            # band 32*(b2%4) of the full 128-partition output via the
            # weight layout (DoubleRow dst must start at partition 0), so
            # 4 pairs accumulate into one PSUM tile.
            SA = psum.tile([128, ROWS], F32, tag="SA")
            SB = psum.tile([128, ROWS], F32, tag="SB")
            # The four per-pair weight variants are overlapping 256-col
            # windows (stride -32) of one band tensor: bands at cols
            # [96,112) (k-tile 0) and [240,256) (k-tile 1) hold the 1/8
            # segment one-hot, so window p sees them at m = 32p+16t+s.
            w_vars = [
                w8[:, 96 - 32 * p : 352 - 32 * p].rearrange("u (t m) -> u t m", t=2)
                for p in range(4)
            ]

            for b2 in range(NPAIR):
                S = SA if b2 < NPAIR // 2 else SB
                p = b2 % 4
                nc.tensor.matmul(
                    out=S[:, :],
                    lhsT=w_vars[p],
                    rhs=x8[:, b2 * CW : (b2 + 1) * CW].rearrange(
                        "u (t n) -> u t n", t=2
                    ),
                    start=(p == 0),
                    stop=(p == 3),
                    perf_mode=mybir.MatmulPerfMode.DoubleRow,
                )

            for k, S in ((0, SA), (1, SB)):
                p = 128
                se = sb.tile([p, NLOC], F32, tag=f"se{k}")
                lnse = sb.tile([p, NLOC], F32, tag=f"lnse{k}")
                dmp = sb.tile([p, ROWS], F32, tag=f"dmp{k}")
                nc.vector.scalar_tensor_tensor(
                    out=dmp[:],
                    in0=S[:],
                    scalar=1.0,
                    in1=oh[:, k * ROWS : (k + 1) * ROWS],
                    op0=mult,
                    op1=mult,
                    accum_out=res[:, k : k + 1],
                )
                E = sb.tile([p, ROWS], BF16, tag=f"E{k}")
                nc.scalar.activation(
                    out=E[:], in_=S[:], func=mybir.ActivationFunctionType.Exp
                )
                nc.vector.reduce_sum(
                    out=se[:],
                    in_=E[:].rearrange("q (i c) -> q i c", c=C),
                    axis=mybir.AxisListType.X,
                )
                nc.scalar.activation(
                    out=lnse[:],
                    in_=se[:],
                    func=mybir.ActivationFunctionType.Ln,
                )
                dms = sb.tile([p, NLOC], F32, tag=f"dms{k}")
                nc.vector.scalar_tensor_tensor(
                    out=dms[:],
                    in0=lnse[:],
                    scalar=1.0,
                    in1=mk[:, k * NLOC : (k + 1) * NLOC],
                    op0=mult,
                    op1=mult,
                    accum_out=res[:, 2 + k : 3 + k],
                )
            nc.scalar.dma_start(out=loss_d[:], in_=res[:])

    nc.finalize()
    return nc


_NC = None


def _get_nc():
    global _NC
    if _NC is None:
        _NC = _build_nc()
    return _NC


def _make_w8():
    import ml_dtypes

    w8 = np.zeros((128, 352), dtype=np.float32)
    u = np.arange(128)
    for s in range(16):
        w8[u // 8 == s, 96 + s] = 1.0 / W
        w8[u // 8 == s, 240 + s] = 1.0 / W
    return w8.astype(ml_dtypes.float8_e4m3fn)


_W8 = None


def make_in_maps(input, target, lab_mask):
    import ml_dtypes

    global _W8
    if _W8 is None:
        _W8 = _make_w8()
    inp = np.asarray(input)
    tgt = np.asarray(target)
    msk = np.asarray(lab_mask)
    in_maps = []
    for cc in range(NCORES):
        xl = inp[cc * NLOC : (cc + 1) * NLOC]                  # [8, 46, 2048]
        xT = xl.transpose(2, 0, 1).reshape(P, ROWS)            # [f, 46*i + c]
        x8 = np.ascontiguousarray(
            xT.reshape(NBLK, 128, ROWS).transpose(1, 0, 2).reshape(128, XCOLS)
        ).astype(ml_dtypes.float8_e4m3fn)
        tl = tgt[cc * NLOC : (cc + 1) * NLOC]                  # [8, 256]
        ml = msk[cc * NLOC : (cc + 1) * NLOC].astype(np.float32)
        ohsel = -(ml[:, :, None] * (tl[:, :, None] == np.arange(C)[None, None, :]))
        ohsel = ohsel.transpose(1, 0, 2).reshape(MP, ROWS)     # [q, 46*i + c]
        mT = ml.T                                              # [q, i]
        oh = np.ascontiguousarray(
            np.concatenate([ohsel[:128], ohsel[128:]], axis=1)
        ).astype(ml_dtypes.bfloat16)
        mk = np.ascontiguousarray(np.concatenate([mT[:128], mT[128:]], axis=1))
        in_maps.append({"x": x8, "w": _W8, "oh": oh, "mk": mk})
    return in_maps


def kernel(input, target, assignment, lab_mask, _trace=False):
    in_maps = make_in_maps(input, target, lab_mask)
    nc = _get_nc()
    for attempt in range(3):
        res = run_bass_kernel_spmd(
            nc, in_maps, core_ids=list(range(NCORES)), trace=_trace
        )
        total = np.float64(0.0)
        for r in res.results:
            total += np.float64(r["loss"].sum())
        if np.isfinite(total):
            break
    out = np.array(total, dtype=np.float32)
    if _trace:
        return out, res
    return out


# revision 63
# speedup vs baseline: 1.0585x; 1.0585x over previous
"""Trainium2 Bass kernel for the segment-reduce masked-CE loss (nn_NewLoss).

Reference math (N=64, C=46, P=2048, MP=256):
    assignment[n, p] = 1 + (p * MP) // P  (contiguous segments of 8 frames)
    pooled[n, q, c]  = mean over the 8 frames of segment q of input[n, c, :]
    loss = -sum_{n,q} lab_mask[n,q] * log_softmax(pooled)[n, q, target[n,q]]

Sharding: data-parallel over batch n across 8 cores (8 items per core);
each core returns per-q partial sums, reduced on the host.

Per-core layout: frames on partitions so the PE does the pooling.
x is shipped fp8_e4m3 as xT[frame, row] (row = 46*item + ch), 16 blocks
of 128 frames; one fp8 DoubleRow matmul per block pair (2 k-tiles), its
32 segments steered into partition band 32*(b2%4) of a full 128-partition
output by the weight layout (DoubleRow dst must start at partition 0), 4
pairs accumulating per PSUM tile S[q, row].  Input DMA is striped across
the three DMA-capable rings (per-ring bandwidth ~60-180 GB/s is the real
limit): scalar carries the weights first (unblocks matmul 0) plus leading
pairs, sync/gpsimd the rest.  Epilogue per tile: picked via
scalar_tensor_tensor accumulate against a host-built masked one-hot
(DVE), EXP (scalar) -> window-46 reduce (DVE) -> Ln (scalar, same act
table as Exp, so no mid-kernel table load) -> mask STT. Host sums the
[128, 4] per-core partials.
"""

import numpy as np

import concourse.bacc as bacc
import concourse.tile as tile
from concourse import mybir
from concourse.bass_utils import run_bass_kernel_spmd

F32 = mybir.dt.float32
BF16 = mybir.dt.bfloat16
F8 = mybir.dt.float8e4

N, C, P, MP = 64, 46, 2048, 256
NCORES = 8
NLOC = N // NCORES            # 8 batch items per core
ROWS = NLOC * C               # 368 (item, channel) rows per core
W = P // MP                   # 8-frame pooling window
NBLK = P // 128               # 16 frame blocks of 128
NPAIR = NBLK // 2             # 8 DoubleRow block pairs
XCOLS = NPAIR * 2 * ROWS      # 5888
CW = 2 * ROWS                 # 736 x cols per pair

# Single combined Exp+Ln activation table: drop Exp/Ln from the per-func
# tables so the fixpoint pass lands on natural_log_exp_and_others and the
# kernel pays only one ACT_TABLE_LOAD (overlapped with the input DMA).
_ORIG_GAT = bacc.get_activation_tables


def _gat_combined(arch):
    exp = mybir.ActivationFunctionType.Exp
    ln = mybir.ActivationFunctionType.Ln
    out = {}
    for name, funcs in _ORIG_GAT(arch).items():
        if name != "natural_log_exp_and_others":
            funcs = funcs - {exp, ln}
        out[name] = funcs
    return out


bacc.get_activation_tables = _gat_combined


def _build_nc():
    nc = bacc.Bacc("TRN2", target_bir_lowering=False)

    x_d = nc.dram_tensor("x", [128, XCOLS], F8, kind="ExternalInput")
    w_d = nc.dram_tensor("w", [128, 352], F8, kind="ExternalInput")
    oh_d = nc.dram_tensor("oh", [128, 2 * ROWS], BF16, kind="ExternalInput")
    mk_d = nc.dram_tensor("mk", [128, 2 * NLOC], F32, kind="ExternalInput")
    loss_d = nc.dram_tensor("loss", [128, 4], F32, kind="ExternalOutput")

    mult = mybir.AluOpType.mult

    with tile.TileContext(nc) as tc:
        with (
            tc.tile_pool(name="xin", bufs=1) as xin,
            tc.tile_pool(name="sb", bufs=1) as sb,
            tc.tile_pool(name="psum", bufs=1, space="PSUM") as psum,
        ):
            # Per-ring DMA bandwidth (~60-110 GB/s) is the real limit, so
            # stripe across all three DMA-capable rings: scalar carries the
            # tiny weights first (unblocks matmul 0) then the leading x
            # pairs, sync and gpsimd carry the rest, oh/mk trail on scalar.
            w8 = sb.tile([128, 352], F8)
            nc.sync.dma_start(out=w8[:], in_=w_d[:])
            x8 = xin.tile([128, XCOLS], F8)
            # one pair per ring in consumption order: each pair has the
            # fewest possible bytes queued ahead of it on its ring, so a
            # slow-phase ring crawl degrades the stream gracefully.
            stripes = [
                (nc.scalar, 0, 1),
                (nc.sync, 1, 2),
                (nc.scalar, 2, 3),
                (nc.gpsimd, 3, 4),
                (nc.sync, 4, 5),
                (nc.scalar, 5, 6),
                (nc.gpsimd, 6, 8),
            ]
            for eng, p0, p1 in stripes:
                eng.dma_start(
                    out=x8[:, p0 * CW : p1 * CW], in_=x_d[:, p0 * CW : p1 * CW]
                )
            oh = sb.tile([128, 2 * ROWS], BF16)
            nc.scalar.dma_start(out=oh[:], in_=oh_d[:])
            mk = sb.tile([128, 2 * NLOC], F32)
            nc.scalar.dma_start(out=mk[:], in_=mk_d[:])

            res = sb.tile([128, 4], F32)

            # DoubleRow pooling: each matmul covers one block pair (2
            # k-tiles of 128 frames); its 32 segments land in partition
            # band 32*(b2%4) of the full 128-partition output via the
            # weight layout (DoubleRow dst must start at partition 0), so
            # 4 pairs accumulate into one PSUM tile.
            SA = psum.tile([128, ROWS], F32, tag="SA")
            SB = psum.tile([128, ROWS], F32, tag="SB")
            # The four per-pair weight variants are overlapping 256-col
            # windows (stride -32) of one band tensor: bands at cols
            # [96,112) (k-tile 0) and [240,256) (k-tile 1) hold the 1/8
            # segment one-hot, so window p sees them at m = 32p+16t+s.
            w_vars = [
                w8[:, 96 - 32 * p : 352 - 32 * p].rearrange("u (t m) -> u t m", t=2)
                for p in range(4)
            ]

            for b2 in range(NPAIR):
                S = SA if b2 < NPAIR // 2 else SB
                p = b2 % 4
                nc.tensor.matmul(
                    out=S[:, :],
                    lhsT=w_vars[p],
                    rhs=x8[:, b2 * CW : (b2 + 1) * CW].rearrange(
                        "u (t n) -> u t n", t=2
                    ),
                    start=(p == 0),
                    stop=(p == 3),
                    perf_mode=mybir.MatmulPerfMode.DoubleRow,
                )

            for k, S in ((0, SA), (1, SB)):
                p = 128
                se = sb.tile([p, NLOC], F32, tag=f"se{k}")
                lnse = sb.tile([p, NLOC], F32, tag=f"lnse{k}")
                dmp = sb.tile([p, ROWS], F32, tag=f"dmp{k}")
                nc.vector.scalar_tensor_tensor(
                    out=dmp[:],
                    in0=S[:],
                    scalar=1.0,
                    in1=oh[:, k * ROWS : (k + 1) * ROWS],
                    op0=mult,
                    op1=mult,
                    accum_out=res[:, k : k + 1],
                )
                E = sb.tile([p, ROWS], BF16, tag=f"E{k}")
                nc.scalar.activation(
                    out=E[:], in_=S[:], func=mybir.ActivationFunctionType.Exp
                )
                nc.vector.reduce_sum(
                    out=se[:],
                    in_=E[:].rearrange("q (i c) -> q i c", c=C),
                    axis=mybir.AxisListType.X,
                )
                nc.scalar.activation(
                    out=lnse[:],
                    in_=se[:],
                    func=mybir.ActivationFunctionType.Ln,
                )
                dms = sb.tile([p, NLOC], F32, tag=f"dms{k}")
                nc.vector.scalar_tensor_tensor(
                    out=dms[:],
                    in0=lnse[:],
                    scalar=1.0,
                    in1=mk[:, k * NLOC : (k + 1) * NLOC],
                    op0=mult,
                    op1=mult,
                    accum_out=res[:, 2 + k : 3 + k],
                )
            nc.scalar.dma_start(out=loss_d[:], in_=res[:])

    nc.finalize()
    return nc


_NC = None


def _get_nc():
    global _NC
    if _NC is None:
        _NC = _build_nc()
    return _NC


def _make_w8():
    import ml_dtypes

    w8 = np.zeros((128, 352), dtype=np.float32)
    u = np.arange(128)
    for s in range(16):
        w8[u // 8 == s, 96 + s] = 1.0 / W
        w8[u // 8 == s, 240 + s] = 1.0 / W
    return w8.astype(ml_dtypes.float8_e4m3fn)


_W8 = None


def make_in_maps(input, target, lab_mask):
    import ml_dtypes

    global _W8
    if _W8 is None:
        _W8 = _make_w8()
    inp = np.asarray(input)
    tgt = np.asarray(target)
    msk = np.asarray(lab_mask)
    in_maps = []
    for cc in range(NCORES):
        xl = inp[cc * NLOC : (cc + 1) * NLOC]                  # [8, 46, 2048]
        xT = xl.transpose(2, 0, 1).reshape(P, ROWS)            # [f, 46*i + c]
        x8 = np.ascontiguousarray(
            xT.reshape(NBLK, 128, ROWS).transpose(1, 0, 2).reshape(128, XCOLS)
        ).astype(ml_dtypes.float8_e4m3fn)
        tl = tgt[cc * NLOC : (cc + 1) * NLOC]                  # [8, 256]
        ml = msk[cc * NLOC : (cc + 1) * NLOC].astype(np.float32)
        ohsel = -(ml[:, :, None] * (tl[:, :, None] == np.arange(C)[None, None, :]))
        ohsel = ohsel.transpose(1, 0, 2).reshape(MP, ROWS)     # [q, 46*i + c]
        mT = ml.T                                              # [q, i]
        oh = np.ascontiguousarray(
            np.concatenate([ohsel[:128], ohsel[128:]], axis=1)
        ).astype(ml_dtypes.bfloat16)
        mk = np.ascontiguousarray(np.concatenate([mT[:128], mT[128:]], axis=1))
        in_maps.append({"x": x8, "w": _W8, "oh": oh, "mk": mk})
    return in_maps


def kernel(input, target, assignment, lab_mask, _trace=False):
    in_maps = make_in_maps(input, target, lab_mask)
    nc = _get_nc()
    for attempt in range(3):
        res = run_bass_kernel_spmd(
            nc, in_maps, core_ids=list(range(NCORES)), trace=_trace
        )
        total = np.float64(0.0)
        for r in res.results:
            total += np.float64(r["loss"].sum())
        if np.isfinite(total):
            break
    out = np.array(total, dtype=np.float32)
    if _trace:
        return out, res
    return out


# revision 64
# speedup vs baseline: 1.1056x; 1.0445x over previous
"""Trainium2 Bass kernel for the segment-reduce masked-CE loss (nn_NewLoss).

Reference math (N=64, C=46, P=2048, MP=256):
    assignment[n, p] = 1 + (p * MP) // P  (contiguous segments of 8 frames)
    pooled[n, q, c]  = mean over the 8 frames of segment q of input[n, c, :]
    loss = -sum_{n,q} lab_mask[n,q] * log_softmax(pooled)[n, q, target[n,q]]

Sharding: data-parallel over batch n across 8 cores (8 items per core);
each core returns per-q partial sums, reduced on the host.

Per-core layout: frames on partitions so the PE does the pooling.
x is shipped fp8_e4m3 as xT[frame, row] (row = 46*item + ch), 16 blocks
of 128 frames; one fp8 DoubleRow matmul per block pair (2 k-tiles), its
32 segments steered into partition band 32*(b2%4) of a full 128-partition
output by the weight layout (DoubleRow dst must start at partition 0), 4
pairs accumulating per PSUM tile S[q, row].  Input DMA is striped across
the three DMA-capable rings (per-ring bandwidth ~60-180 GB/s is the real
limit): scalar carries the weights first (unblocks matmul 0) plus leading
pairs, sync/gpsimd the rest.  Epilogue per tile: picked via
scalar_tensor_tensor accumulate against a host-built masked one-hot
(DVE), EXP (scalar) -> window-46 reduce (DVE) -> Ln (scalar, same act
table as Exp, so no mid-kernel table load) -> mask STT. Host sums the
[128, 4] per-core partials.
"""

import numpy as np

import concourse.bacc as bacc
import concourse.tile as tile
from concourse import mybir
from concourse.bass_utils import run_bass_kernel_spmd

F32 = mybir.dt.float32
BF16 = mybir.dt.bfloat16
F8 = mybir.dt.float8e4
I8 = mybir.dt.int8

N, C, P, MP = 64, 46, 2048, 256
NCORES = 8
NLOC = N // NCORES            # 8 batch items per core
ROWS = NLOC * C               # 368 (item, channel) rows per core
W = P // MP                   # 8-frame pooling window
NBLK = P // 128               # 16 frame blocks of 128
NPAIR = NBLK // 2             # 8 DoubleRow block pairs
XCOLS = NPAIR * 2 * ROWS      # 5888
CW = 2 * ROWS                 # 736 x cols per pair

# Single combined Exp+Ln activation table: drop Exp/Ln from the per-func
# tables so the fixpoint pass lands on natural_log_exp_and_others and the
# kernel pays only one ACT_TABLE_LOAD (overlapped with the input DMA).
_ORIG_GAT = bacc.get_activation_tables


def _gat_combined(arch):
    exp = mybir.ActivationFunctionType.Exp
    ln = mybir.ActivationFunctionType.Ln
    out = {}
    for name, funcs in _ORIG_GAT(arch).items():
        if name != "natural_log_exp_and_others":
            funcs = funcs - {exp, ln}
        out[name] = funcs
    return out


bacc.get_activation_tables = _gat_combined


def _build_nc():
    nc = bacc.Bacc("TRN2", target_bir_lowering=False)

    x_d = nc.dram_tensor("x", [128, XCOLS], F8, kind="ExternalInput")
    w_d = nc.dram_tensor("w", [128, 352], F8, kind="ExternalInput")
    oh_d = nc.dram_tensor("oh", [128, 2 * ROWS], I8, kind="ExternalInput")
    mk_d = nc.dram_tensor("mk", [128, 2 * NLOC], F32, kind="ExternalInput")
    loss_d = nc.dram_tensor("loss", [128, 4], F32, kind="ExternalOutput")

    mult = mybir.AluOpType.mult

    with tile.TileContext(nc) as tc:
        with (
            tc.tile_pool(name="xin", bufs=1) as xin,
            tc.tile_pool(name="sb", bufs=1) as sb,
            tc.tile_pool(name="psum", bufs=1, space="PSUM") as psum,
        ):
            # Per-ring DMA bandwidth (~60-110 GB/s) is the real limit, so
            # stripe across all three DMA-capable rings: scalar carries the
            # tiny weights first (unblocks matmul 0) then the leading x
            # pairs, sync and gpsimd carry the rest, oh/mk trail on scalar.
            w8 = sb.tile([128, 352], F8)
            nc.sync.dma_start(out=w8[:], in_=w_d[:])
            x8 = xin.tile([128, XCOLS], F8)
            # one pair per ring in consumption order: each pair has the
            # fewest possible bytes queued ahead of it on its ring, so a
            # slow-phase ring crawl degrades the stream gracefully.
            stripes = [
                (nc.scalar, 0, 1),
                (nc.sync, 1, 2),
                (nc.scalar, 2, 3),
                (nc.gpsimd, 3, 4),
                (nc.sync, 4, 5),
                (nc.scalar, 5, 6),
                (nc.gpsimd, 6, 8),
            ]
            for eng, p0, p1 in stripes:
                eng.dma_start(
                    out=x8[:, p0 * CW : p1 * CW], in_=x_d[:, p0 * CW : p1 * CW]
                )
            oh = sb.tile([128, 2 * ROWS], I8)
            nc.scalar.dma_start(out=oh[:], in_=oh_d[:])
            mk = sb.tile([128, 2 * NLOC], F32)
            nc.scalar.dma_start(out=mk[:], in_=mk_d[:])

            res = sb.tile([128, 4], F32)

            # DoubleRow pooling: each matmul covers one block pair (2
            # k-tiles of 128 frames); its 32 segments land in partition
            # band 32*(b2%4) of the full 128-partition output via the
            # weight layout (DoubleRow dst must start at partition 0), so
            # 4 pairs accumulate into one PSUM tile.
            SA = psum.tile([128, ROWS], F32, tag="SA")
            SB = psum.tile([128, ROWS], F32, tag="SB")
            # The four per-pair weight variants are overlapping 256-col
            # windows (stride -32) of one band tensor: bands at cols
            # [96,112) (k-tile 0) and [240,256) (k-tile 1) hold the 1/8
            # segment one-hot, so window p sees them at m = 32p+16t+s.
            w_vars = [
                w8[:, 96 - 32 * p : 352 - 32 * p].rearrange("u (t m) -> u t m", t=2)
                for p in range(4)
            ]

            for b2 in range(NPAIR):
                S = SA if b2 < NPAIR // 2 else SB
                p = b2 % 4
                nc.tensor.matmul(
                    out=S[:, :],
                    lhsT=w_vars[p],
                    rhs=x8[:, b2 * CW : (b2 + 1) * CW].rearrange(
                        "u (t n) -> u t n", t=2
                    ),
                    start=(p == 0),
                    stop=(p == 3),
                    perf_mode=mybir.MatmulPerfMode.DoubleRow,
                )

            for k, S in ((0, SA), (1, SB)):
                p = 128
                se = sb.tile([p, NLOC], F32, tag=f"se{k}")
                lnse = sb.tile([p, NLOC], F32, tag=f"lnse{k}")
                dmp = sb.tile([p, ROWS], F32, tag=f"dmp{k}")
                nc.vector.scalar_tensor_tensor(
                    out=dmp[:],
                    in0=S[:],
                    scalar=1.0,
                    in1=oh[:, k * ROWS : (k + 1) * ROWS],
                    op0=mult,
                    op1=mult,
                    accum_out=res[:, k : k + 1],
                )
                E = sb.tile([p, ROWS], BF16, tag=f"E{k}")
                nc.scalar.activation(
                    out=E[:], in_=S[:], func=mybir.ActivationFunctionType.Exp
                )
                nc.vector.reduce_sum(
                    out=se[:],
                    in_=E[:].rearrange("q (i c) -> q i c", c=C),
                    axis=mybir.AxisListType.X,
                )
                nc.scalar.activation(
                    out=lnse[:],
                    in_=se[:],
                    func=mybir.ActivationFunctionType.Ln,
                )
                dms = sb.tile([p, NLOC], F32, tag=f"dms{k}")
                nc.vector.scalar_tensor_tensor(
                    out=dms[:],
                    in0=lnse[:],
                    scalar=1.0,
                    in1=mk[:, k * NLOC : (k + 1) * NLOC],
                    op0=mult,
                    op1=mult,
                    accum_out=res[:, 2 + k : 3 + k],
                )
            nc.scalar.dma_start(out=loss_d[:], in_=res[:])

    nc.finalize()
    return nc


_NC = None


def _get_nc():
    global _NC
    if _NC is None:
        _NC = _build_nc()
    return _NC


def _make_w8():
    import ml_dtypes

    w8 = np.zeros((128, 352), dtype=np.float32)
    u = np.arange(128)
    for s in range(16):
        w8[u // 8 == s, 96 + s] = 1.0 / W
        w8[u // 8 == s, 240 + s] = 1.0 / W
    return w8.astype(ml_dtypes.float8_e4m3fn)


_W8 = None


def make_in_maps(input, target, lab_mask):
    import ml_dtypes

    global _W8
    if _W8 is None:
        _W8 = _make_w8()
    inp = np.asarray(input)
    tgt = np.asarray(target)
    msk = np.asarray(lab_mask)
    in_maps = []
    for cc in range(NCORES):
        xl = inp[cc * NLOC : (cc + 1) * NLOC]                  # [8, 46, 2048]
        xT = xl.transpose(2, 0, 1).reshape(P, ROWS)            # [f, 46*i + c]
        x8 = np.ascontiguousarray(
            xT.reshape(NBLK, 128, ROWS).transpose(1, 0, 2).reshape(128, XCOLS)
        ).astype(ml_dtypes.float8_e4m3fn)
        tl = tgt[cc * NLOC : (cc + 1) * NLOC]                  # [8, 256]
        ml = msk[cc * NLOC : (cc + 1) * NLOC].astype(np.float32)
        ohsel = -(ml[:, :, None] * (tl[:, :, None] == np.arange(C)[None, None, :]))
        ohsel = ohsel.transpose(1, 0, 2).reshape(MP, ROWS)     # [q, 46*i + c]
        mT = ml.T                                              # [q, i]
        oh = np.ascontiguousarray(
            np.concatenate([ohsel[:128], ohsel[128:]], axis=1)
        ).astype(np.int8)
        mk = np.ascontiguousarray(np.concatenate([mT[:128], mT[128:]], axis=1))
        in_maps.append({"x": x8, "w": _W8, "oh": oh, "mk": mk})
    return in_maps


def kernel(input, target, assignment, lab_mask, _trace=False):
    in_maps = make_in_maps(input, target, lab_mask)
    nc = _get_nc()
    for attempt in range(3):
        res = run_bass_kernel_spmd(
            nc, in_maps, core_ids=list(range(NCORES)), trace=_trace
        )
        total = np.float64(0.0)
        for r in res.results:
            total += np.float64(r["loss"].sum())
        if np.isfinite(total):
            break
    out = np.array(total, dtype=np.float32)
    if _trace:
        return out, res
    return out
